# revision 1
# baseline (speedup 1.0000x reference)
"""Contour-to-mask winding-number kernel for 8 Trainium2 NeuronCores.

Problem: for each of 16 contours (64 vertices each) and each pixel of a
128x128 grid, sum over polygon edges k:
    tanh(1e5*cross_k) * acos(clip(dot_k / (|d_k||rd_k|), -1+eps, 1-eps))
then |sum| / 2pi clipped to [0, 1].

Key identity used on device: with the clip mapped through monotone acos,
    summand = tanh(1e5*cross) * clip(acos_angle, amin, amax)
    acos_angle = atan(|cross/dot|)            if dot >= 0
               = pi - atan(|cross/dot|)       if dot <  0
which needs no norms / sqrt at all (verified offline: max abs diff vs the
jax reference 1e-5, L2 rel 1.3e-6).

Layout per core (2 contours): SBUF partition p = contour*64 + edge k,
free dim = pixel (i major, j minor; 16384 pixels). Since the grid factors
as m = (i/128, j/128), diff-x quantities depend only on (p, i) and diff-y
quantities only on (p, j) -> all first-stage arrays are tiny [128,128]
tiles combined into full [128,16384] arrays via free-dim stride-0
broadcast views. The edge sum is a TensorE matmul against a 0/1 mask
(scaled by 1/2pi) contracting the 128 partitions to the 2 contours.
"""

import math

import numpy as np

B, N, KV, S = 2, 8, 64, 128
S2 = S * S
NCON = B * N
NCORES = 8
CPC = NCON // NCORES  # contours per core

CHUNK = 4096  # pixels per full-size tile
NCHUNK = S2 // CHUNK
IBLK = CHUNK // S  # i values per chunk

EPS = 1e-5
AMIN = float(np.float32(math.acos(1.0 - EPS)))
AMAX = float(np.float32(math.acos(-1.0 + EPS)))
PI = float(np.float32(math.pi))
INV2PI = float(np.float32(1.0 / (2.0 * math.pi)))
K_SIGN = 1.0e5

# which engine runs each splittable full-size op ("vector" | "gpsimd")
ENG_T2 = "gpsimd"
ENG_DOT = "gpsimd"
ENG_SMD = "gpsimd"

_CACHE = {}


# --------------------------------------------------------------------------
# workaround: walrus rejects instructions carrying many sem waits; Tile's
# exit drain waits on every used semaphore.  Split across several drains.
def _patch_tile_drain():
    import bass_rust
    import concourse.tile as tile

    if getattr(tile.TileContext, "_ctm_drain_patched", False):
        return
    MAX_WAITS = 1

    def _drain_and_barrier(self, tick_clock, wait_clock):
        from concourse.vector_clock import ScopedClock

        nc = self.nc
        drain_inst = nc.sync.drain()
        wait_clock.add_sem_waits(
            drain_inst.ins, ScopedClock({None: tick_clock.global_clock})
        )
        si = drain_inst.ins.sync_info
        if si is not None and len(si.on_wait) > MAX_WAITS:
            waits = list(si.on_wait)
            drain_inst.ins.sync_info = bass_rust.SyncInfo(
                on_wait=waits[:MAX_WAITS], on_update=list(si.on_update)
            )
            for off in range(MAX_WAITS, len(waits), MAX_WAITS):
                extra = nc.sync.drain()
                extra.ins.sync_info = bass_rust.SyncInfo(
                    on_wait=waits[off : off + MAX_WAITS], on_update=[]
                )
        nc.all_engine_barrier()
        popped = nc._tile_sem_poison_stack.pop()
        assert popped is self._sem_poison
        nc.clear_and_free_semaphores(list(self.sems.allocated().values()))
        nc.all_engine_barrier()

    tile.TileContext._drain_and_barrier = _drain_and_barrier
    tile.TileContext._ctm_drain_patched = True


def _split_sync_waits(nc, max_waits=1):
    """Walrus codegen rejects instructions carrying more than a couple of sem
    waits.  Move excess waits onto same-engine NOPs inserted just before."""
    import bass_rust

    n = 0
    for fn in nc.m.functions:
        for blk in fn.blocks:
            insts = blk.instructions
            out = []
            for inst in insts:
                si = inst.sync_info
                waits = list(si.on_wait) if si is not None else []
                if len(waits) > max_waits:
                    for off in range(max_waits, len(waits), max_waits):
                        nop = bass_rust.InstNoOp(name=f"ctm_waitnop_{n}", ins=[], outs=[])
                        n += 1
                        nop.engine = inst.engine
                        nop.sync_info = bass_rust.SyncInfo(
                            on_wait=waits[off : off + max_waits], on_update=[]
                        )
                        out.append(nop)
                    inst.sync_info = bass_rust.SyncInfo(
                        on_wait=waits[:max_waits], on_update=list(si.on_update)
                    )
                out.append(inst)
            if n:
                blk.instructions = out
    return n


def _act_raw(nc, out, in_, func, bias=0.0, scale=1.0, alpha=0.0):
    """Emit InstActivation directly.  Needed for Reciprocal, which bass's
    Python wrapper refuses; measured accuracy here is ~1.2e-5 max rel over
    1e-8..1e8, ample for cross/dot (feeds arctan)."""
    import concourse.mybir as mybir

    se = nc.scalar
    ins = [se.lower_ap(in_)]
    for arg in (bias, scale, alpha):
        ins.append(mybir.ImmediateValue(dtype=mybir.dt.float32, value=float(arg)))
    return se.add_instruction(
        mybir.InstActivation(
            name=nc.get_next_instruction_name(),
            func=func,
            ins=ins,
            outs=[se.lower_ap(out)],
        )
    )


# --------------------------------------------------------------------------
def _build_bass(repeat=1):
    """Build the per-core Bass module (identical on all 8 cores).

    repeat>1 re-runs the whole compute that many times (same tiles) —
    used only for slope-based HW timing in test.py."""
    from contextlib import ExitStack

    import concourse.bass as bass
    import concourse.mybir as mybir
    import concourse.tile as tile

    _patch_tile_drain()
    F32 = mybir.dt.float32
    AF = mybir.ActivationFunctionType
    Alu = mybir.AluOpType

    nc = bass.Bass()
    cparams = nc.dram_tensor("cparams", [128, 8], F32, kind="ExternalInput")
    ngrid = nc.dram_tensor("ngrid", [128, S], F32, kind="ExternalInput")
    maskw = nc.dram_tensor("maskw", [128, CPC], F32, kind="ExternalInput")
    out = nc.dram_tensor("out", [CPC, S2], F32, kind="ExternalOutput")

    def eng(which):
        return nc.gpsimd if which == "gpsimd" else nc.vector

    with tile.TileContext(nc) as tc, ExitStack() as ctx:
        const = ctx.enter_context(tc.tile_pool(name="const", bufs=1))
        poolA = ctx.enter_context(tc.tile_pool(name="A", bufs=2))
        poolB = ctx.enter_context(tc.tile_pool(name="Bp", bufs=2))
        poolC = ctx.enter_context(tc.tile_pool(name="C", bufs=2))
        poolD = ctx.enter_context(tc.tile_pool(name="D", bufs=2))
        psum = ctx.enter_context(tc.tile_pool(name="ps", bufs=8, space="PSUM"))

        cp = const.tile([128, 8], F32)
        nc.sync.dma_start(cp[:], cparams[:])
        ng = const.tile([128, S], F32)
        nc.sync.dma_start(ng[:], ngrid[:])
        mw = const.tile([128, CPC], F32)
        nc.sync.dma_start(mw[:], maskw[:])

        # tiny per-edge tiles: dx[p,i] = cx[p] - i/128 etc (ngrid = -x/128)
        dx_t = const.tile([128, S], F32)
        nc.vector.tensor_scalar(out=dx_t[:], in0=ng[:], scalar1=cp[:, 0:1],
                                scalar2=None, op0=Alu.add)
        dy_t = const.tile([128, S], F32)
        nc.vector.tensor_scalar(out=dy_t[:], in0=ng[:], scalar1=cp[:, 1:2],
                                scalar2=None, op0=Alu.add)
        rdx_t = const.tile([128, S], F32)
        nc.vector.tensor_scalar(out=rdx_t[:], in0=ng[:], scalar1=cp[:, 2:3],
                                scalar2=None, op0=Alu.add)
        rdy_t = const.tile([128, S], F32)
        nc.vector.tensor_scalar(out=rdy_t[:], in0=ng[:], scalar1=cp[:, 3:4],
                                scalar2=None, op0=Alu.add)
        dxrdx = const.tile([128, S], F32)
        nc.vector.tensor_tensor(out=dxrdx[:], in0=dx_t[:], in1=rdx_t[:], op=Alu.mult)
        dyrdy = const.tile([128, S], F32)
        nc.vector.tensor_tensor(out=dyrdy[:], in0=dy_t[:], in1=rdy_t[:], op=Alu.mult)

        red_sb = const.tile([CPC, S2], F32)  # per-contour winding sums, |.| applied
        final = const.tile([128, CPC * S], F32)

        for ch in range(NCHUNK * repeat):
            ch = ch % NCHUNK
            i0 = ch * IBLK
            sh3 = [128, IBLK, S]

            def bj(t):  # broadcast a [128, S] j-tile over the i axis
                return t[:].unsqueeze(1).broadcast_to(sh3)

            def bi(t):  # broadcast this chunk's i-slice over the j axis
                return t[:, i0 : i0 + IBLK].unsqueeze(2).broadcast_to(sh3)

            a = poolA.tile([128, CHUNK], F32)
            b = poolB.tile([128, CHUNK], F32)
            c = poolC.tile([128, CHUNK], F32)
            d = poolD.tile([128, CHUNK], F32)
            a3 = a[:].rearrange("p (x y) -> p x y", x=IBLK)
            b3 = b[:].rearrange("p (x y) -> p x y", x=IBLK)
            c3 = c[:].rearrange("p (x y) -> p x y", x=IBLK)

            # a = cross = dy*rdx - dx*rdy ;  c = dot = dx*rdx + dy*rdy
            nc.vector.tensor_tensor(out=a3, in0=bj(dy_t), in1=bi(rdx_t), op=Alu.mult)
            eng(ENG_T2).tensor_tensor(out=b3, in0=bj(rdy_t), in1=bi(dx_t), op=Alu.mult)
            eng(ENG_DOT).tensor_tensor(out=c3, in0=bi(dxrdx), in1=bj(dyrdy), op=Alu.add)
            nc.vector.tensor_tensor(out=a[:], in0=a[:], in1=b[:], op=Alu.subtract)

            # d = atan(|cross| * (1/dot))  -- carries sign(dot);
            # angle = d + pi*[dot<0]; clip; * tanh(1e5*cross)
            _act_raw(nc, d[:], c[:], AF.Reciprocal)
            nc.vector.scalar_tensor_tensor(out=b[:], in0=a[:], scalar=-1.0,
                                           in1=a[:], op0=Alu.mult, op1=Alu.max)
            nc.vector.tensor_tensor(out=d[:], in0=b[:], in1=d[:], op=Alu.mult)
            nc.scalar.activation(d[:], d[:], AF.Arctan)
            nc.vector.tensor_scalar(out=c[:], in0=c[:], scalar1=0.0, scalar2=PI,
                                    op0=Alu.is_lt, op1=Alu.mult)
            eng(ENG_SMD).tensor_tensor(out=d[:], in0=d[:], in1=c[:], op=Alu.add)
            nc.vector.tensor_scalar(out=d[:], in0=d[:], scalar1=AMIN, scalar2=AMAX,
                                    op0=Alu.max, op1=Alu.min)
            nc.scalar.activation(a[:], a[:], AF.Tanh, 0.0, K_SIGN)
            eng(ENG_SMD).tensor_tensor(out=d[:], in0=d[:], in1=a[:], op=Alu.mult)

            # edge-sum via PE: [128,2] mask (x 1/2pi) contracts partitions;
            # ScalarE Abs moves PSUM->SBUF and applies |.| in one pass
            for m in range(CHUNK // 512):
                ps = psum.tile([CPC, 512], F32)
                nc.tensor.matmul(ps[:], mw[:], d[:, m * 512 : (m + 1) * 512],
                                 start=True, stop=True)
                px0 = ch * CHUNK + m * 512
                nc.scalar.activation(red_sb[:, px0 : px0 + 512], ps[:], AF.Abs)

        # redistribute [2, 16384] onto 128 partitions (i on partitions)
        for cc in range(CPC):
            nc.sync.dma_start(final[:, cc * S : (cc + 1) * S], red_sb[cc : cc + 1, :])
        nc.vector.tensor_scalar(out=final[:], in0=final[:], scalar1=1.0,
                                scalar2=None, op0=Alu.min)
        nc.sync.dma_start(out[:].rearrange("c (i j) -> i c j", i=S), final[:])

    _split_sync_waits(nc)
    return nc


def _get_nc():
    if "nc" not in _CACHE:
        _CACHE["nc"] = _build_bass()
    return _CACHE["nc"]


def _make_in_maps(contour):
    c = contour.reshape(NCON, KV, 2)
    g = np.arange(S, dtype=np.float32) / np.float32(S)
    ngrid_np = np.ascontiguousarray(np.broadcast_to(-g[None, :], (128, S)))
    maskw_np = np.zeros((128, CPC), np.float32)
    for lc in range(CPC):
        maskw_np[lc * KV : (lc + 1) * KV, lc] = INV2PI
    in_maps = []
    for core in range(NCORES):
        cp = np.zeros((128, 8), np.float32)
        for lc in range(CPC):
            cq = c[core * CPC + lc]
            cp[lc * KV : (lc + 1) * KV, 0] = cq[:, 0]
            cp[lc * KV : (lc + 1) * KV, 1] = cq[:, 1]
            cp[lc * KV : (lc + 1) * KV, 2] = np.roll(cq[:, 0], -1)
            cp[lc * KV : (lc + 1) * KV, 3] = np.roll(cq[:, 1], -1)
        in_maps.append({"cparams": cp, "ngrid": ngrid_np, "maskw": maskw_np})
    return in_maps


def kernel(contour, size):
    contour = np.asarray(contour, dtype=np.float32)
    size = int(size)
    assert contour.shape == (B, N, KV, 2), contour.shape
    assert size == S, size

    from concourse.bass_utils import run_bass_kernel_spmd

    nc = _get_nc()
    in_maps = _make_in_maps(contour)
    res = run_bass_kernel_spmd(nc, in_maps, core_ids=list(range(NCORES)))
    full = np.concatenate([res.results[i]["out"] for i in range(NCORES)], axis=0)
    return full.reshape(B, N, S, S).astype(np.float32)

